# revision 6
# baseline (speedup 1.0000x reference)
"""DenseGAT Trainium2 kernel (8 NeuronCores, batch-parallel).

Math: per (batch, head):
  h = x @ W.T ; a_src[i] = h[i]*att_src ; a_dst[j] = h[j]*att_dst
  s_ij = a_src[i] + a_dst[j] ; P = adj * exp(leakyrelu_0.2(s))
  out[i] = (P @ h)[i] / sum_j P[i,j]

Key identity: exp(lrelu(s)) = [s>=0]*u_i*v_j + [s<0]*p_i*q_j with
  u = exp(a_src), v = exp(a_dst), p = exp(0.2 a_src), q = exp(0.2 a_dst).
With adjH = adj * [s>=0] (built in ONE fused DVE op per tile-row:
scalar_tensor_tensor (bcast >= -a_dst) * adjT) and rq = q*[h|1],
rnv = [-q*[h|1] | v*[h|1]]:
  out_unnorm = (adjT@rq + adjH@(-rq)) + r .* (adjH@rv),  r = u/p = exp(0.8 a_src)
(the whole row is scaled by 1/p_i vs the raw scores, which cancels in the
softmax normalization). Column 64 of the aug matmuls is the denominator.

Each core handles one batch sample (B=8 across 8 cores).
"""

import numpy as np

import concourse.bass as bass
import concourse.mybir as mybir
import concourse.tile as tile
from concourse import bacc
from concourse.bass import broadcast_tensor_aps
from concourse.bass_utils import run_bass_kernel_spmd
from concourse.masks import make_identity

P = 128
B, L, CIN, COUT, HEADS = 8, 2048, 256, 256, 4
HD = COUT // HEADS          # 64
NT = L // P                 # 16 tiles along L
KB = CIN // P               # 2 chunks along cin/cout
NEG = 0.2
N_CORES = 8

F32 = mybir.dt.float32
BF16 = mybir.dt.bfloat16
U8 = mybir.dt.uint8
F8 = mybir.dt.float8e4
AF = mybir.ActivationFunctionType
OP = mybir.AluOpType

_NC_CACHE = {}

NAUG = HD + 1               # 65
RPAD = HD + 2               # 66 (padded row length for rq)
HALF = L // 2


def _build():
    nc = bacc.Bacc(None, target_bir_lowering=False, debug=False)
    x_in = nc.declare_dram_parameter("x", [L, CIN], F32, isOutput=False)
    adj_in = nc.declare_dram_parameter("adj", [L, L], U8, isOutput=False)
    w_in = nc.declare_dram_parameter("W", [COUT, CIN], F32, isOutput=False)
    asrc_in = nc.declare_dram_parameter("att_src", [1, HEADS, 1, HD], F32, isOutput=False)
    adst_in = nc.declare_dram_parameter("att_dst", [1, HEADS, 1, HD], F32, isOutput=False)
    out_d = nc.declare_dram_parameter("out", [L, COUT], F32, isOutput=True)

    with tile.TileContext(nc) as tc:
        with (
            tc.tile_pool(name="const", bufs=1) as cpool,
            tc.tile_pool(name="big", bufs=1) as big,
        ):
            ident_f8 = cpool.tile([P, P], F8)
            make_identity(nc, ident_f8)
            ident_bf = cpool.tile([P, P], BF16)
            make_identity(nc, ident_bf)
            ones_bf = cpool.tile([1, P], BF16)
            nc.vector.memset(ones_bf[:], 1.0)

            # persistent big tensors
            adjT = big.tile([P, NT, L], BF16)          # adj transposed, j on partitions
            h_bf = big.tile([P, NT, COUT], BF16)       # h natural (L on partitions)
            a_bf = big.tile([8, L], BF16)              # 2H score rows bf16
            a_cols = big.tile([P, NT, 8], F32)         # transposed score columns
            bcast4 = big.tile([P, HEADS, L], BF16)     # a_src bcast per head
            # per-head per-node-column factors [P, HEADS, NT, 1]
            rcol = big.tile([P, HEADS, NT, 1], F32)    # exp(0.8 a_src)
            vcol = big.tile([P, HEADS, NT, 1], F32)    # exp(a_dst)
            qcol = big.tile([P, HEADS, NT, 1], F32)    # exp(0.2 a_dst)
            nqcol = big.tile([P, HEADS, NT, 1], F32)   # -exp(0.2 a_dst)
            nadst = big.tile([P, HEADS, NT, 1], F32)   # -a_dst
            rq4 = big.tile([P, HEADS, NT, RPAD], BF16)   # [q*h | q | pad]
            rnv4 = big.tile([P, HEADS, NT, 2 * RPAD], BF16)  # [-qh|-q|pad|vh|v|pad]

            # ---------------- prep ----------------
            with (
                tc.tile_pool(name="adj_nat", bufs=3) as anat_pool,
                tc.tile_pool(name="xload", bufs=3) as xload,
                tc.tile_pool(name="big2", bufs=1) as big2,
                tc.tile_pool(name="adj_ps", bufs=2, space="PSUM") as aps_pool,
                tc.tile_pool(name="prep_ps", bufs=2, space="PSUM") as pps,
                tc.tile_pool(name="p2k_ps", bufs=2, space="PSUM") as sps,
            ):
                xT_bf = big2.tile([P, KB, L], BF16)        # x^T bf16 (cin on partitions)
                w_nat = big2.tile([P, KB, CIN], F32)       # W natural (cout on partitions)
                w_bf = big2.tile([P, KB, CIN], BF16)
                wT_bf = big2.tile([P, KB, COUT], BF16)     # W^T (cin on partitions)
                attW = big2.tile([P, KB, 2 * HEADS], F32)  # [cout, 2H] att matrix
                attc = big2.tile([P, KB, 2 * HEADS], F32)  # (W^T @ attW): [cin, 2H]
                attc_bf = big2.tile([P, KB, 2 * HEADS], BF16)
                a_all = big2.tile([8, L], F32)             # 2H score rows

                # W natural + attW (DMA only, early)
                nc.sync.dma_start(
                    out=w_nat[:], in_=w_in[:].rearrange("(kb p) c -> p kb c", p=P)
                )
                nc.vector.memset(attW[:], 0.0)
                for h in range(HEADS):
                    cb, prow = divmod(HD * h, P)
                    nc.sync.dma_start(
                        out=attW[prow : prow + HD, cb, 2 * h : 2 * h + 1],
                        in_=asrc_in[0, h, 0, :].rearrange("(d one) -> d one", one=1),
                    )
                    nc.sync.dma_start(
                        out=attW[prow : prow + HD, cb, 2 * h + 1 : 2 * h + 2],
                        in_=adst_in[0, h, 0, :].rearrange("(d one) -> d one", one=1),
                    )

                # x^T in bf16 (feeds score path and h)
                for c in range(NT):
                    xn = xload.tile([P, CIN], F32, name="xn")
                    nc.sync.dma_start(out=xn[:], in_=x_in[c * P : (c + 1) * P, :])
                    xb = xload.tile([P, CIN], BF16, name="xb")
                    nc.scalar.activation(xb[:], xn[:], AF.Copy, bias=0.0, scale=1.0)
                    xp = pps.tile([P, KB, P], BF16, tag="prep")
                    for kb in range(KB):
                        nc.tensor.transpose(
                            xp[:, kb, :], xb[:, kb * P : (kb + 1) * P], ident_bf[:]
                        )
                    nc.vector.tensor_copy(xT_bf[:, :, c * P : (c + 1) * P], xp[:])

                # score path: attc = W^T @ attW, a = attc^T @ x^T
                for mb in range(KB):
                    ap_ps = pps.tile([P, 2 * HEADS], F32, tag="prep")
                    for cb in range(KB):
                        nc.tensor.matmul(
                            ap_ps[:], w_nat[:, cb, mb * P : (mb + 1) * P], attW[:, cb, :],
                            start=(cb == 0), stop=(cb == KB - 1),
                        )
                    nc.scalar.activation(attc[:, mb, :], ap_ps[:], AF.Copy, bias=0.0, scale=1.0)
                nc.vector.tensor_copy(attc_bf[:], attc[:])

                for nb in range(4):
                    a_ps = sps.tile([8, 512], F32, tag="big")
                    for kb in range(KB):
                        nc.tensor.matmul(
                            a_ps[:], attc_bf[:, kb, :], xT_bf[:, kb, nb * 512 : (nb + 1) * 512],
                            start=(kb == 0), stop=(kb == KB - 1),
                        )
                    nc.scalar.activation(
                        a_all[:, nb * 512 : (nb + 1) * 512], a_ps[:], AF.Copy, bias=0.0, scale=1.0
                    )
                nc.vector.tensor_copy(a_bf[:], a_all[:])

                for t in range(NT):
                    acp = pps.tile([P, 8], BF16, tag="prep")
                    nc.tensor.transpose(
                        acp[:], a_bf[0:8, t * P : (t + 1) * P], ident_bf[0:8, 0:8]
                    )
                    nc.scalar.activation(a_cols[:, t, :], acp[:], AF.Copy, bias=0.0, scale=1.0)

                # per-head column factors (ACT) + a_src broadcast rows (PE + ACT)
                for h in range(HEADS):
                    asl = a_cols[:, :, 2 * h : 2 * h + 1]
                    adl = a_cols[:, :, 2 * h + 1 : 2 * h + 2]
                    nc.scalar.activation(rcol[:, h], asl, AF.Exp, bias=0.0, scale=0.8)
                    nc.scalar.activation(vcol[:, h], adl, AF.Exp, bias=0.0, scale=1.0)
                    nc.scalar.activation(qcol[:, h], adl, AF.Exp, bias=0.0, scale=NEG)
                    nc.vector.tensor_scalar(
                        out=nqcol[:, h], in0=qcol[:, h], scalar1=-1.0, scalar2=None, op0=OP.mult
                    )
                    nc.vector.tensor_scalar(
                        out=nadst[:, h], in0=adl, scalar1=-1.0, scalar2=None, op0=OP.mult
                    )

                for h in range(HEADS):
                    arow = big2.tile([1, L], BF16, tag="arow", name="arow")
                    nc.sync.dma_start(out=arow[:], in_=a_bf[2 * h : 2 * h + 1, :])
                    for nb in range(4):
                        bps = sps.tile([P, 512], F32, tag="big")
                        nc.tensor.matmul(
                            bps[:], ones_bf[:], arow[0:1, nb * 512 : (nb + 1) * 512],
                            start=True, stop=True,
                        )
                        nc.scalar.activation(
                            bcast4[:, h, nb * 512 : (nb + 1) * 512], bps[:],
                            AF.Copy, bias=0.0, scale=1.0,
                        )

                # adjacency: load, transpose (fp8 PE), evacuate (ACT/DVE split)
                def adj_tile(c):
                    an = anat_pool.tile([P, L], U8, name="an")
                    nc.sync.dma_start(out=an[:], in_=adj_in[c * P : (c + 1) * P, :])
                    an_f8 = an[:].bitcast(F8)
                    tp = aps_pool.tile([P, NT, P, 2], F8, tag="tp", name="tp")
                    for t in range(NT):
                        nc.tensor.transpose(
                            tp[:, t, :, 0], an_f8[:, t * P : (t + 1) * P], ident_f8[:]
                        )
                    # fp8 0x01 = 2^-9; scale 512 -> exact 1.0 in bf16.
                    if c % 2 == 0:
                        nc.scalar.activation(
                            adjT[:, :, c * P : (c + 1) * P], tp[:, :, :, 0],
                            AF.Copy, bias=0.0, scale=512.0,
                        )
                    else:
                        nc.vector.tensor_scalar(
                            out=adjT[:, :, c * P : (c + 1) * P], in0=tp[:, :, :, 0],
                            scalar1=512.0, scalar2=None, op0=OP.mult,
                        )

                for c in range(NT // 2):
                    adj_tile(c)

                # W^T then h = x @ W.T
                nc.scalar.activation(w_bf[:], w_nat[:], AF.Copy, bias=0.0, scale=1.0)
                for cb in range(KB):
                    wp = pps.tile([P, KB, P], BF16, tag="prep")
                    for ib in range(KB):
                        nc.tensor.transpose(
                            wp[:, ib, :], w_bf[:, cb, ib * P : (ib + 1) * P], ident_bf[:]
                        )
                    for ib in range(KB):
                        nc.scalar.activation(
                            wT_bf[:, ib, cb * P : (cb + 1) * P], wp[:, ib, :],
                            AF.Copy, bias=0.0, scale=1.0,
                        )
                for c in range(NT):
                    hp = sps.tile([P, COUT], F32, tag="big")
                    for kb in range(KB):
                        nc.tensor.matmul(
                            hp[:], xT_bf[:, kb, c * P : (c + 1) * P], wT_bf[:, kb, :],
                            start=(kb == 0), stop=(kb == KB - 1),
                        )
                    nc.scalar.activation(h_bf[:, c, :], hp[:], AF.Copy, bias=0.0, scale=1.0)

                # rq / rnv for all heads (gpsimd, broadcast multiply) + aug columns
                for h in range(HEADS):
                    hsrc = h_bf[:, :, h * HD : (h + 1) * HD]       # [P, NT, HD]
                    for dst, col in (
                        (rq4[:, h, :, 0:HD], qcol[:, h]),
                        (rnv4[:, h, :, 0:HD], nqcol[:, h]),
                        (rnv4[:, h, :, RPAD : RPAD + HD], vcol[:, h]),
                    ):
                        i0, i1 = broadcast_tensor_aps(hsrc, col)
                        nc.gpsimd.tensor_tensor(out=dst, in0=i0, in1=i1, op=OP.mult)
                    nc.vector.tensor_copy(rq4[:, h, :, HD : HD + 1], qcol[:, h])
                    nc.vector.tensor_copy(rnv4[:, h, :, HD : HD + 1], nqcol[:, h])
                    nc.vector.tensor_copy(
                        rnv4[:, h, :, RPAD + HD : RPAD + HD + 1], vcol[:, h]
                    )

                for c in range(NT // 2, NT):
                    adj_tile(c)

            # ---------------- per-head attention ----------------
            with (
                tc.tile_pool(name="adjH", bufs=2) as adjHp,
                tc.tile_pool(name="es", bufs=4) as esp,
                tc.tile_pool(name="sall", bufs=2) as sallp,
                tc.tile_pool(name="outst", bufs=2) as outp,
                tc.tile_pool(name="mm_ps", bufs=3, space="PSUM") as mmps,
            ):

                def build_head(h):
                    # adjH = (bcast >= -a_dst) * adjT, one fused op per (t, half)
                    halves = []
                    for half in range(2):
                        i0 = half * HALF
                        adjH = adjHp.tile([P, NT, HALF], BF16, tag="adjH", name="adjH")
                        halves.append(adjH)
                        for t in range(NT):
                            nc.vector.scalar_tensor_tensor(
                                out=adjH[:, t, :],
                                in0=bcast4[:, h, i0 : i0 + HALF],
                                scalar=nadst[:, h, t],
                                in1=adjT[:, t, i0 : i0 + HALF],
                                op0=OP.is_ge,
                                op1=OP.mult,
                            )
                    return halves

                def compute_head(h, halves):
                    out_stage = outp.tile([P, NT, HD], F32, tag="outst")
                    s_all = sallp.tile([P, NT, NAUG], F32, tag="s_all")
                    rnv_h = rnv4[:, h].rearrange("p t (b c) -> p t b c", b=2)
                    for half in range(2):
                        i0 = half * HALF
                        adjH = halves[half]
                        for ic in range(HALF // P):
                            cg = half * (HALF // P) + ic
                            isl = slice(i0 + ic * P, i0 + (ic + 1) * P)
                            hsl = slice(ic * P, (ic + 1) * P)
                            po = mmps.tile([P, 2 * NAUG], F32, tag="po", bufs=6)
                            for t in range(NT):
                                # adjH @ [-rq | rv] -> cols 0:130 (first starts the bank)
                                nc.tensor.matmul(
                                    po[:].rearrange("p (b c) -> p b c", b=2),
                                    adjH[:, t, hsl], rnv_h[:, t, :, 0:NAUG],
                                    start=(t == 0), stop=(t == NT - 1),
                                    skip_group_check=True,
                                )
                                # adjT @ rq accumulates into cols 0:65
                                nc.tensor.matmul(
                                    po[:, 0:NAUG], adjT[:, t, isl], rq4[:, h, t, 0:NAUG],
                                    start=False, stop=(t == NT - 1),
                                    skip_group_check=True,
                                )
                            # es = r * po[65:130]  (ACT, PSUM->SBUF with row scale)
                            es = esp.tile([P, NAUG], F32, tag="es", bufs=4)
                            nc.scalar.activation(
                                es[:], po[:, NAUG : 2 * NAUG], AF.Identity,
                                bias=0.0, scale=rcol[:, h, cg],
                            )
                            # s_all = es + po[0:65]  (DVE, one PSUM operand)
                            nc.vector.tensor_tensor(
                                out=s_all[:, cg, :], in0=es[:], in1=po[:, 0:NAUG], op=OP.add
                            )

                    rall = esp.tile([P, NT], F32, tag="rall", bufs=2)
                    nc.vector.reciprocal(
                        rall[:], s_all[:, :, HD : HD + 1].rearrange("p t one -> p (t one)")
                    )
                    for cg in range(NT):
                        nc.scalar.activation(
                            out_stage[:, cg, :], s_all[:, cg, 0:HD], AF.Identity,
                            bias=0.0, scale=rall[:, cg : cg + 1],
                        )
                    nc.gpsimd.dma_start(
                        out=out_d[:].rearrange("(c p) (hh d) -> p c hh d", p=P, d=HD)[:, :, h, :],
                        in_=out_stage[:],
                    )

                prev = None
                for h in range(HEADS):
                    halves = build_head(h)
                    if prev is not None:
                        compute_head(h - 1, prev)
                    prev = halves
                compute_head(HEADS - 1, prev)

    nc.finalize()
    return nc


def kernel(x, adj_mask, W, att_src, att_dst):
    if "nc" not in _NC_CACHE:
        _NC_CACHE["nc"] = _build()
    nc = _NC_CACHE["nc"]

    x = np.ascontiguousarray(np.asarray(x, dtype=np.float32))
    W = np.ascontiguousarray(np.asarray(W, dtype=np.float32))
    att_src = np.ascontiguousarray(np.asarray(att_src, dtype=np.float32))
    att_dst = np.ascontiguousarray(np.asarray(att_dst, dtype=np.float32))
    adj = np.ascontiguousarray(adj_mask).view(np.uint8)

    in_maps = [
        {
            "x": x[b],
            "adj": adj[b],
            "W": W,
            "att_src": att_src,
            "att_dst": att_dst,
        }
        for b in range(N_CORES)
    ]
    res = run_bass_kernel_spmd(nc, in_maps, core_ids=list(range(N_CORES)))
    out = np.stack([res.results[b]["out"] for b in range(N_CORES)], axis=0)
    return out.astype(np.float32)
